# revision 14
# baseline (speedup 1.0000x reference)
"""MiniAttention Trainium2 Bass kernel.

Problem: B=8, N=1024, C=768, H=12, D=64.
  qkv = x @ w_qkv.T ; q,k,v heads ; S = (q*SCALE) @ k.T per head
  A1 = conv_l-mix over heads ; P = softmax_m(A1) ; A2 = conv_w-mix over heads
  out = (A2 @ v per head) @ w_proj.T + b_proj
Sharding: pure batch-parallel, 1 batch element per NeuronCore (8 cores).

Per-core design (PE matmuls in bf16, f32 accumulation):
  - Host passes x^T, w_qkv^T, w_proj^T (transposed on host, bf16).
  - Scores per head h evac'd into S_all [block_row, (h, m)]; head-interleave
    into groups of G=10 queries with row map r = rr*12 + h via ONE SWDGE
    DMA per group (engines execute in order, and HWDGE dma_start costs
    ~600ns of Sync occupancy each, so both queue choice and instruction
    count matter).
  - conv_l (SCALE folded) as constant rr-block-diagonal lhsT; exp on ACT
    with accum_out giving softmax sums; normalization folded into the
    per-group conv_w lhsT (rows scaled by 1/sum, built on GpSimd).
  - A2 -> xbar DMA-transpose -> attention@V contracts m at K=128 with
    column-packed head pairs (full 128-partition PSUM tile).
  - All engines execute their instruction streams IN ORDER, so the block
    loop is software-pipelined by emission order: per iteration emit
    mixes(b-1) with a 3-half skew (mix2 lags mix1 so ACT exp latency is
    hidden) and the AV chains of block b-2 injected between mix steps,
    then scores(b) last. PSUM: scores+mix1 share one 4-buffer ring
    (tag "ps"), mix2 2 banks, AV 2 banks = 8 banks total.
  - PSUM->SBUF evacs are greedily balanced between ACT and DVE by
    accumulated estimated cost (exp is pinned to ACT, small ops to GpSimd).
"""

import numpy as np
import ml_dtypes

B, N, C, H = 8, 1024, 768, 12
D = C // H
SCALE = D ** -0.5
G = 10          # queries per mix group
NB = 120        # queries per block (12 groups)
NBLK = 8        # full blocks; last block is ragged: 6 groups of 10 + 1 of 4
BF16 = ml_dtypes.bfloat16

_cached = None


def _block_layout():
    """Returns list of blocks: (n0, nb, chunks) where chunks is a list of
    (row_start, g_start, g_count, g_size) describing the query groups."""
    blocks = []
    for b in range(NBLK):
        blocks.append((b * NB, NB, [(0, 0, 12, G)]))
    # ragged tail: n in [960, 1024) = 6 groups of 10 + 1 group of 4
    blocks.append((960, 64, [(0, 0, 6, G), (60, 6, 1, 4)]))
    return blocks


def _build_program():
    import concourse.tile as tile
    from concourse import bacc, mybir

    f32 = mybir.dt.float32
    bf16 = mybir.dt.bfloat16
    Exp = mybir.ActivationFunctionType.Exp

    nc = bacc.Bacc("TRN2", target_bir_lowering=False, debug=False)

    xt = nc.dram_tensor("xt", [C, N], bf16, kind="ExternalInput").ap()
    wqkvt = nc.dram_tensor("wqkvt", [C, 3 * C], bf16, kind="ExternalInput").ap()
    wprojt = nc.dram_tensor("wprojt", [C, C], bf16, kind="ExternalInput").ap()
    m1w_in = nc.dram_tensor("m1w", [12 * G, 128], bf16, kind="ExternalInput").ap()
    m2p_in = nc.dram_tensor("m2p", [128, 128], f32, kind="ExternalInput").ap()
    out_d = nc.dram_tensor("out", [C, N], f32, kind="ExternalOutput").ap()

    KC = C // 128  # 6 contraction chunks

    # greedy ACT/DVE balance by estimated occupancy (ns)
    _load = {"act": 0.0, "dve": 0.0}

    def evac(dst, src, n):
        """PSUM->SBUF copy of [rows, n]; pick the less-loaded engine."""
        if _load["act"] + 200 + n / 1.2 <= _load["dve"] + 200 + n / 0.96:
            _load["act"] += 200 + n / 1.2
            nc.scalar.copy(dst, src)
        else:
            _load["dve"] += 200 + n / 0.96
            nc.vector.tensor_copy(dst, src)

    with tile.TileContext(nc) as tc:
        with tc.tile_pool(name="const", bufs=1) as const, \
             tc.tile_pool(name="big", bufs=1) as big:

            m1wsb = const.tile([120, 128], bf16)
            nc.sync.dma_start(m1wsb, m1w_in)
            m2psb = const.tile([128, 128], f32)
            nc.sync.dma_start(m2psb, m2p_in)

            # persistent activations
            qksb = big.tile([128, 2 * KC, N], bf16)   # ct 0..5 = q, 6..11 = k
            vsb = big.tile([128, 8, C], bf16)         # [m%128, m//128, cout]
            attnT = big.tile([128, KC, N], bf16)      # [cout2%128, cout2//128, n]

            # ---------------- QKV ----------------
            with tc.tile_pool(name="xtp", bufs=1) as xtp, \
                 tc.tile_pool(name="qkvps", bufs=3, space="PSUM") as qkvps, \
                 tc.tile_pool(name="vps", bufs=2, space="PSUM") as vps:
                xtsb = xtp.tile([128, KC, N], bf16)
                nc.sync.dma_start(xtsb, xt.rearrange("(kc p) n -> p kc n", p=128))
                wqsb = xtp.tile([128, KC, 3 * C], bf16)
                nc.sync.dma_start(
                    wqsb, wqkvt.rearrange("(kc p) c -> p kc c", p=128))

                # q, k: out[cout_tile, n]
                for ct in range(12):
                    for nh in range(2):
                        ps = qkvps.tile([128, 512], f32, tag="qkv")
                        for kc in range(KC):
                            nc.tensor.matmul(
                                ps,
                                lhsT=wqsb[:, kc, 128 * ct:128 * ct + 128],
                                rhs=xtsb[:, kc, 512 * nh:512 * nh + 512],
                                start=(kc == 0), stop=(kc == KC - 1),
                            )
                        evac(qksb[:, ct, 512 * nh:512 * nh + 512], ps, 512)

                # v: out[n_tile, cout]
                for nt in range(8):
                    ps = vps.tile([128, 768], f32, tag="vps")
                    for half, (c0, c1) in enumerate([(0, 512), (512, 768)]):
                        for kc in range(KC):
                            nc.tensor.matmul(
                                ps[:, c0:c1],
                                lhsT=xtsb[:, kc, 128 * nt:128 * nt + 128],
                                rhs=wqsb[:, kc, 2 * C + c0:2 * C + c1],
                                start=(kc == 0), stop=(kc == KC - 1),
                            )
                    evac(vsb[:, nt, :], ps, 768)

            # ------- attention: software-pipelined over blocks -------
            # Per iteration (emission order == execution order per engine):
            #   1. dense PE burst: AV chains of block b-2 woven with score
            #      matmul pairs of block b (keeps the PE busy enough for the
            #      HAM clock-gate to hold 2.4 GHz; score evacs drain on
            #      ACT/DVE underneath),
            #   2. interleave DMAs of block b (sync/gpsimd alternating),
            #   3. mix steps of block b-1 (mix2T lags mix1 by SKEW to hide
            #      the ACT exp latency).
            # Mix matmuls carry 128 weight columns so the compiler enables
            # FWL (4x faster LDWEIGHTS): m1w is column-padded with zeros, so
            # a1 rows [rows:128) are written as zeros, exp turns them into
            # finite 1s, and m2p's zero rows [120:128) zero them out of A2.
            blocks = _block_layout()
            nblk = len(blocks)

            with tc.tile_pool(name="sallp", bufs=1) as sallp, \
                 tc.tile_pool(name="sintp", bufs=3) as sintp, \
                 tc.tile_pool(name="pintp", bufs=3) as pintp, \
                 tc.tile_pool(name="a2tp", bufs=2) as a2tp, \
                 tc.tile_pool(name="smp", bufs=8) as smp, \
                 tc.tile_pool(name="m2wp", bufs=6) as m2wp, \
                 tc.tile_pool(name="psmix", bufs=2, space="PSUM") as psmix, \
                 tc.tile_pool(name="psA2", bufs=1, space="PSUM") as psA2, \
                 tc.tile_pool(name="psAV", bufs=2, space="PSUM") as psAV:

                sint_t = {}
                a2t_t = {}
                _dq = [0]

                def s_thunks(bi):
                    """Scores of block bi: 12 per-head thunks (2 MMs into a
                    2-bank psum tile + one evac), then the interleave DMAs."""
                    n0, nb, chunks = blocks[bi]
                    sall = sallp.tile([128, 12, N], bf16, tag="sall",
                                      name="sall")
                    out = []

                    def mk(h):
                        def emit():
                            base = 64 * (h % 2)
                            ps = psmix.tile([128, 1024], f32, tag="ps",
                                            name="ps")
                            for mh in range(2):
                                nc.tensor.matmul(
                                    ps[0:nb, 512 * mh:512 * mh + 512],
                                    lhsT=qksb[base:base + 64, h // 2,
                                              n0:n0 + nb],
                                    rhs=qksb[base:base + 64, 6 + h // 2,
                                             512 * mh:512 * mh + 512],
                                    start=True, stop=True,
                                )
                            evac(sall[0:nb, h, :], ps[0:nb, :], 1024)
                        return emit

                    for h in range(12):
                        out.append(mk(h))

                    def interleave():
                        # sint[rr*12+h, g, m] = S_all[g*gs+rr, h, m]
                        sint = sintp.tile([128, 12, N], bf16, tag="sint",
                                          name="sint")
                        sint_t[bi] = sint
                        for (rs, g0, gc, gs) in chunks:
                            for gi in range(gc):
                                eng = nc.sync if _dq[0] % 2 == 0 else nc.gpsimd
                                _dq[0] += 1
                                eng.dma_start(
                                    out=sint[0:12 * gs, g0 + gi, :],
                                    in_=sall[rs + gi * gs:
                                             rs + (gi + 1) * gs, :, :],
                                )
                    out.append(interleave)
                    return out

                def stage_M_gen(bi):
                    """Mixes of block bi at group steps; mix2 (transposed,
                    straight into a2t layout) lags mix1 by SKEW steps."""
                    n0, nb, chunks = blocks[bi]
                    sint = sint_t.pop(bi)
                    a2t = a2tp.tile([128, 12, 8, 128], bf16, tag="a2t",
                                    name="a2t")
                    a2t_t[bi] = a2t
                    steps = []
                    for (rs, g0, gc, gs) in chunks:
                        for g in range(g0, g0 + gc):
                            steps.append((g, gs))
                    st = {}

                    def mix1(g, gs):
                        rows = 12 * gs
                        pg = pintp.tile([128, N], bf16, tag="pint", name="pg")
                        sm = smp.tile([128, 2], f32, tag="sm", name="sm")
                        st[g] = {"pg": pg, "sm": sm, "w2": None, "gs": gs}
                        a1 = psmix.tile([128, 1024], f32, tag="ps", name="a1")
                        for mh in range(2):
                            nc.tensor.matmul(
                                a1[:, 512 * mh:512 * mh + 512],
                                lhsT=m1wsb[0:rows, 0:128],
                                rhs=sint[0:rows, g, 512 * mh:512 * mh + 512],
                                start=True, stop=True,
                            )
                        _load["act"] += 1050
                        nc.scalar.activation(
                            pg, a1, Exp,
                            accum_out=sm[:, 0:1],
                        )
                        _load["dve"] += 150
                        nc.vector.reciprocal(sm[:, 1:2], sm[:, 0:1])
                        w2 = m2wp.tile([128, 128], bf16, tag="m2w", name="w2")
                        st[g]["w2"] = w2
                        kr = 128 if gs == G else 12 * gs
                        nc.gpsimd.tensor_scalar_mul(
                            w2[0:kr, :], m2psb[0:kr, :], sm[0:kr, 1:2])

                    def mix2T(g, gs):
                        # A2^T chunk: out[m', (rr,o)] = sum_rows
                        #   pg[row, m'] * w2[row, (rr,o)] -- m on partitions,
                        # written directly into the a2t (transposed) layout.
                        # K=128 (padded rows contribute exp(0)*0) -> FWL.
                        s = st.pop(g)
                        kr = 128 if gs == G else 12 * gs
                        a2 = psA2.tile([128, 8, 128], f32, tag="psA2",
                                       name="a2")
                        for c in range(8):
                            nc.tensor.matmul(
                                a2[:, c, :],
                                lhsT=s["pg"][0:kr, 128 * c:128 * c + 128],
                                rhs=s["w2"][0:kr, :],
                                start=True, stop=True,
                            )
                            if c == 3:
                                evac(a2t[:, g, 0:4, :], a2[:, 0:4, :], 512)
                        evac(a2t[:, g, 4:8, :], a2[:, 4:8, :], 512)

                    SKEW = 2
                    for i in range(len(steps) + SKEW):
                        if i < len(steps):
                            mix1(*steps[i])
                        if i >= SKEW:
                            mix2T(*steps[i - SKEW])
                        yield

                def av_thunks(bi):
                    """attention@V of block bi: 6 column-packed head-pair
                    chains accumulating into shared 1-bank psum tiles."""
                    n0, nb, chunks = blocks[bi]
                    a2t = a2t_t.pop(bi)
                    out = []
                    tiles = {}

                    def mk(j):
                        def emit():
                            grp, jj = divmod(j, 4)
                            npair = 4 if grp == 0 else 2
                            if jj == 0:
                                tiles[grp] = psAV.tile(
                                    [128, npair, 128], f32, tag="psAV",
                                    name="av", padded_shape=[128, 4, 128])
                            av = tiles[grp]
                            for half in range(2):
                                o = 2 * j + half
                                for (rs, g0, gc, gs) in chunks:
                                    for c in range(8):
                                        nc.tensor.matmul(
                                            av[64 * half:64 * half + 64, jj,
                                               rs:rs + gc * gs],
                                            lhsT=vsb[:, c, 64 * o:64 * o + 64],
                                            rhs=a2t[:, g0:g0 + gc, c,
                                                    o:o + 12 * (gs - 1) + 1:12],
                                            start=(c == 0), stop=(c == 7),
                                        )
                            if jj == npair - 1:
                                evac(attnT[:, 4 * grp:4 * grp + npair,
                                           n0:n0 + nb],
                                     av[:, :, 0:nb], npair * nb)
                        return emit

                    for j in range(6):
                        out.append(mk(j))
                    return out

                for it in range(nblk + 3):
                    avs = av_thunks(it - 3) if 3 <= it <= nblk + 2 else []
                    ss = s_thunks(it) if it < nblk else []
                    # 1. dense PE burst: AV chains woven with score pairs
                    si = 0
                    for t in avs:
                        t()
                        for _ in range(2):
                            if si < len(ss) - 1:
                                ss[si]()
                                si += 1
                    while si < len(ss) - 1:
                        ss[si]()
                        si += 1
                    # 2. interleave DMAs of block `it`
                    if ss:
                        ss[-1]()
                    # 3. mixes of block it-2 (two-iteration skew: the
                    #    interleave DMAs get a full iteration to land)
                    if 2 <= it <= nblk + 1:
                        for _ in stage_M_gen(it - 2):
                            pass

            # ---------------- proj ----------------
            with tc.tile_pool(name="projps", bufs=3, space="PSUM") as pjp, \
                 tc.tile_pool(name="outp", bufs=3) as outp:
                wpsb = outp.tile([128, KC, C], bf16)
                nc.sync.dma_start(
                    wpsb, wprojt.rearrange("(kc p) c -> p kc c", p=128))
                od = out_d.rearrange("(ct p) n -> p ct n", p=128)
                for ct in range(KC):
                    for nh in range(2):
                        ps = pjp.tile([128, 512], f32, tag="pj")
                        for kc in range(KC):
                            nc.tensor.matmul(
                                ps,
                                lhsT=wpsb[:, kc, 128 * ct:128 * ct + 128],
                                rhs=attnT[:, kc, 512 * nh:512 * nh + 512],
                                start=(kc == 0), stop=(kc == KC - 1),
                            )
                        ob = outp.tile([128, 512], f32, tag="ob")
                        evac(ob, ps, 512)
                        nc.sync.dma_start(
                            od[:, ct, 512 * nh:512 * nh + 512], ob)

    nc.compile()
    return nc


def _mix_weights(conv_l_w, conv_w_w):
    """Host-built mix lhsT matrices, row map r = rr*12 + h.

    m1w[rr*12+h, rr*12+o] = SCALE * conv_l[o, h]   (lhsT for mix1)
    m2p[rr*12+h, rr*12+o] = conv_w[o, h]           (pattern, f32; scaled
        per-group on device by 1/softmax_sum per row; 128 cols, cols >= 120
        are zero so A2 psum rows [rows:128) are zeros)
    The gs=4 ragged group uses the leading [48, 48] / [48, :] slices.
    """
    m1 = np.zeros((120, 128), np.float32)
    m2 = np.zeros((128, 128), np.float32)
    for rr in range(G):
        for h in range(12):
            for o in range(12):
                m1[rr * 12 + h, rr * 12 + o] = SCALE * conv_l_w[o, h]
                m2[rr * 12 + h, rr * 12 + o] = conv_w_w[o, h]
    return m1.astype(BF16), m2.astype(np.float32)


def _run(x, w_qkv, w_proj, b_proj, conv_l_w, conv_w_w, **spmd_kwargs):
    global _cached
    from concourse import bass_utils

    x = np.asarray(x, np.float32)
    w_qkv = np.asarray(w_qkv, np.float32)
    w_proj = np.asarray(w_proj, np.float32)
    b_proj = np.asarray(b_proj, np.float32)
    conv_l_w = np.asarray(conv_l_w, np.float32)
    conv_w_w = np.asarray(conv_w_w, np.float32)

    if _cached is None:
        _cached = _build_program()
    nc = _cached

    m1w, m2p = _mix_weights(conv_l_w, conv_w_w)
    wqkvt = np.ascontiguousarray(w_qkv.T).astype(BF16)
    wprojt = np.ascontiguousarray(w_proj.T).astype(BF16)

    in_maps = []
    for b in range(B):
        in_maps.append({
            "xt": np.ascontiguousarray(x[b].T).astype(BF16),
            "wqkvt": wqkvt,
            "wprojt": wprojt,
            "m1w": m1w,
            "m2p": m2p,
        })

    res = bass_utils.run_bass_kernel_spmd(
        nc, in_maps, core_ids=list(range(B)), **spmd_kwargs)
    out = np.stack([res.results[b]["out"].T for b in range(B)])  # [B, N, C]
    return (out + b_proj[None, None, :]).astype(np.float32), res


def kernel(x, w_qkv, w_proj, b_proj, conv_l_w, conv_w_w):
    out, _ = _run(x, w_qkv, w_proj, b_proj, conv_l_w, conv_w_w)
    return out


# revision 17
# speedup vs baseline: 1.0939x; 1.0939x over previous
"""MiniAttention Trainium2 Bass kernel.

Problem: B=8, N=1024, C=768, H=12, D=64.
  qkv = x @ w_qkv.T ; q,k,v heads ; S = (q*SCALE) @ k.T per head
  A1 = conv_l-mix over heads ; P = softmax_m(A1) ; A2 = conv_w-mix over heads
  out = (A2 @ v per head) @ w_proj.T + b_proj
Sharding: pure batch-parallel, 1 batch element per NeuronCore (8 cores).

Per-core design (PE matmuls in bf16, f32 accumulation):
  - Host passes x^T, w_qkv^T, w_proj^T (transposed on host, bf16).
  - Scores per head h evac'd into S_all [block_row, (h, m)]; head-interleave
    into groups of G=10 queries with row map r = rr*12 + h via ONE SWDGE
    DMA per group (engines execute in order, and HWDGE dma_start costs
    ~600ns of Sync occupancy each, so both queue choice and instruction
    count matter).
  - conv_l (SCALE folded) as constant rr-block-diagonal lhsT; exp on ACT
    with accum_out giving softmax sums; normalization folded into the
    per-group conv_w lhsT (rows scaled by 1/sum, built on GpSimd).
  - A2 -> xbar DMA-transpose -> attention@V contracts m at K=128 with
    column-packed head pairs (full 128-partition PSUM tile).
  - All engines execute their instruction streams IN ORDER, so the block
    loop is software-pipelined by emission order: per iteration emit
    mixes(b-1) with a 3-half skew (mix2 lags mix1 so ACT exp latency is
    hidden) and the AV chains of block b-2 injected between mix steps,
    then scores(b) last. PSUM: scores+mix1 share one 4-buffer ring
    (tag "ps"), mix2 2 banks, AV 2 banks = 8 banks total.
  - PSUM->SBUF evacs are greedily balanced between ACT and DVE by
    accumulated estimated cost (exp is pinned to ACT, small ops to GpSimd).
"""

import numpy as np
import ml_dtypes

B, N, C, H = 8, 1024, 768, 12
D = C // H
SCALE = D ** -0.5
G = 10          # queries per mix group
NB = 120        # queries per block (12 groups)
NBLK = 8        # full blocks; last block is ragged: 6 groups of 10 + 1 of 4
BF16 = ml_dtypes.bfloat16

_cached = None


def _block_layout():
    """Returns list of blocks: (n0, nb, chunks) where chunks is a list of
    (row_start, g_start, g_count, g_size) describing the query groups."""
    blocks = []
    for b in range(NBLK):
        blocks.append((b * NB, NB, [(0, 0, 12, G)]))
    # ragged tail: n in [960, 1024) = 8 groups of 8
    blocks.append((960, 64, [(0, 0, 8, 8)]))
    return blocks


def _build_program():
    import concourse.tile as tile
    from concourse import bacc, mybir

    f32 = mybir.dt.float32
    bf16 = mybir.dt.bfloat16
    Exp = mybir.ActivationFunctionType.Exp

    nc = bacc.Bacc("TRN2", target_bir_lowering=False, debug=False)

    xt = nc.dram_tensor("xt", [C, N], bf16, kind="ExternalInput").ap()
    wqkvt = nc.dram_tensor("wqkvt", [C, 3 * C], bf16, kind="ExternalInput").ap()
    wprojt = nc.dram_tensor("wprojt", [C, C], bf16, kind="ExternalInput").ap()
    m1w_in = nc.dram_tensor("m1w", [12 * G, 128], bf16, kind="ExternalInput").ap()
    m2p_in = nc.dram_tensor("m2p", [128, 128], f32, kind="ExternalInput").ap()
    out_d = nc.dram_tensor("out", [C, N], f32, kind="ExternalOutput").ap()

    KC = C // 128  # 6 contraction chunks

    # greedy ACT/DVE balance by estimated occupancy (ns)
    _load = {"act": 0.0, "dve": 0.0}

    def evac(dst, src, n):
        """PSUM->SBUF copy of [rows, n]; pick the less-loaded engine."""
        if _load["act"] + 200 + n / 1.2 <= _load["dve"] + 200 + n / 0.96:
            _load["act"] += 200 + n / 1.2
            nc.scalar.copy(dst, src)
        else:
            _load["dve"] += 200 + n / 0.96
            nc.vector.tensor_copy(dst, src)

    with tile.TileContext(nc) as tc:
        with tc.tile_pool(name="const", bufs=1) as const, \
             tc.tile_pool(name="big", bufs=1) as big:

            m1wsb = const.tile([120, 128], bf16)
            nc.sync.dma_start(m1wsb, m1w_in)
            m2psb = const.tile([128, 128], f32)
            nc.sync.dma_start(m2psb, m2p_in)

            # persistent activations
            qksb = big.tile([128, 2 * KC, N], bf16)   # ct 0..5 = q, 6..11 = k
            vsb = big.tile([128, 8, C], bf16)         # [m%128, m//128, cout]
            attnT = big.tile([128, KC, N], bf16)      # [cout2%128, cout2//128, n]

            # ---------------- QKV ----------------
            with tc.tile_pool(name="xtp", bufs=1) as xtp, \
                 tc.tile_pool(name="qkvps", bufs=3, space="PSUM") as qkvps, \
                 tc.tile_pool(name="vps", bufs=2, space="PSUM") as vps:
                xtsb = xtp.tile([128, KC, N], bf16)
                nc.sync.dma_start(xtsb, xt.rearrange("(kc p) n -> p kc n", p=128))
                wqsb = xtp.tile([128, KC, 3 * C], bf16)
                nc.sync.dma_start(
                    wqsb, wqkvt.rearrange("(kc p) c -> p kc c", p=128))

                # q, k: out[cout_tile, n]
                for ct in range(12):
                    for nh in range(2):
                        ps = qkvps.tile([128, 512], f32, tag="qkv")
                        for kc in range(KC):
                            nc.tensor.matmul(
                                ps,
                                lhsT=wqsb[:, kc, 128 * ct:128 * ct + 128],
                                rhs=xtsb[:, kc, 512 * nh:512 * nh + 512],
                                start=(kc == 0), stop=(kc == KC - 1),
                            )
                        evac(qksb[:, ct, 512 * nh:512 * nh + 512], ps, 512)

                # v: out[n_tile, cout]
                for nt in range(8):
                    ps = vps.tile([128, 768], f32, tag="vps")
                    for half, (c0, c1) in enumerate([(0, 512), (512, 768)]):
                        for kc in range(KC):
                            nc.tensor.matmul(
                                ps[:, c0:c1],
                                lhsT=xtsb[:, kc, 128 * nt:128 * nt + 128],
                                rhs=wqsb[:, kc, 2 * C + c0:2 * C + c1],
                                start=(kc == 0), stop=(kc == KC - 1),
                            )
                    evac(vsb[:, nt, :], ps, 768)

            # ------- attention: software-pipelined over blocks -------
            # Per iteration (emission order == execution order per engine):
            #   1. dense PE burst: AV chains of block b-2 woven with score
            #      matmul pairs of block b (keeps the PE busy enough for the
            #      HAM clock-gate to hold 2.4 GHz; score evacs drain on
            #      ACT/DVE underneath),
            #   2. interleave DMAs of block b (sync/gpsimd alternating),
            #   3. mix steps of block b-1 (mix2T lags mix1 by SKEW to hide
            #      the ACT exp latency).
            # Mix matmuls carry 128 weight columns so the compiler enables
            # FWL (4x faster LDWEIGHTS): m1w is column-padded with zeros, so
            # a1 rows [rows:128) are written as zeros, exp turns them into
            # finite 1s, and m2p's zero rows [120:128) zero them out of A2.
            blocks = _block_layout()
            nblk = len(blocks)

            with tc.tile_pool(name="sallp", bufs=1) as sallp, \
                 tc.tile_pool(name="sintp", bufs=2) as sintp, \
                 tc.tile_pool(name="pintp", bufs=4) as pintp, \
                 tc.tile_pool(name="a2tp", bufs=2) as a2tp, \
                 tc.tile_pool(name="smp", bufs=8) as smp, \
                 tc.tile_pool(name="m2wp", bufs=6) as m2wp, \
                 tc.tile_pool(name="psmix", bufs=2, space="PSUM") as psmix, \
                 tc.tile_pool(name="psA2", bufs=1, space="PSUM") as psA2, \
                 tc.tile_pool(name="psAV", bufs=2, space="PSUM") as psAV:

                sint_t = {}
                a2t_t = {}
                _dq = [0]

                def s_thunks(bi):
                    """Scores of block bi: 12 per-head thunks (2 MMs into a
                    2-bank psum tile + one evac), then the interleave DMAs."""
                    n0, nb, chunks = blocks[bi]
                    sall = sallp.tile([128, 12, N], bf16, tag="sall",
                                      name="sall")
                    out = []

                    (rs0, g00, gc, gs) = chunks[0]

                    def mk(h):
                        def emit():
                            base = 64 * (h % 2)
                            ps = psmix.tile([128, 1024], f32, tag="ps",
                                            name="ps")
                            # x columns are block-locally permuted on the
                            # host so that psum row rr*gc+g holds query
                            # n0+g*gs+rr: the interleave below then reads a
                            # stride-gc partition slice (spreads across all
                            # SDMA engine ports), and the permutation
                            # cancels at the AV stage.
                            for mh in range(2):
                                nc.tensor.matmul(
                                    ps[0:nb, 512 * mh:512 * mh + 512],
                                    lhsT=qksb[base:base + 64, h // 2,
                                              n0:n0 + nb],
                                    rhs=qksb[base:base + 64, 6 + h // 2,
                                             512 * mh:512 * mh + 512],
                                    start=True, stop=True,
                                )
                            evac(sall[0:nb, h, :], ps[0:nb, :], 1024)
                        return emit

                    for h in range(12):
                        out.append(mk(h))

                    def interleave():
                        # sint[rr*12+h, g, m] = sall[rr*gc+g, h, m]
                        sint = sintp.tile([128, 12, N], bf16, tag="sint",
                                          name="sint")
                        sint_t[bi] = sint
                        for g in range(gc):
                            eng = nc.sync if _dq[0] % 2 == 0 else nc.gpsimd
                            _dq[0] += 1
                            eng.dma_start(
                                out=sint[0:12 * gs, g, :],
                                in_=sall[g:gc * gs:gc, :, :],
                            )
                    out.append(interleave)
                    return out

                def stage_M_gen(bi):
                    """Mixes of block bi at group steps; mix2 (transposed,
                    straight into a2t layout) lags mix1 by SKEW steps."""
                    n0, nb, chunks = blocks[bi]
                    sint = sint_t.pop(bi)
                    a2t = a2tp.tile([128, 12, 8, 128], bf16, tag="a2t",
                                    name="a2t")
                    a2t_t[bi] = a2t
                    steps = []
                    for (rs, g0, gc, gs) in chunks:
                        for g in range(g0, g0 + gc):
                            steps.append((g, gs))
                    st = {}

                    def mix1(g, gs):
                        rows = 12 * gs
                        pg = pintp.tile([128, N], bf16, tag="pint", name="pg")
                        sm = smp.tile([128, 2], f32, tag="sm", name="sm")
                        st[g] = {"pg": pg, "sm": sm, "w2": None, "gs": gs}
                        a1 = psmix.tile([128, 1024], f32, tag="ps", name="a1")
                        for mh in range(2):
                            nc.tensor.matmul(
                                a1[:, 512 * mh:512 * mh + 512],
                                lhsT=m1wsb[0:rows, 0:128],
                                rhs=sint[0:rows, g, 512 * mh:512 * mh + 512],
                                start=True, stop=True,
                            )
                        _load["act"] += 1050
                        nc.scalar.activation(
                            pg, a1, Exp,
                            accum_out=sm[:, 0:1],
                        )
                        _load["dve"] += 150
                        nc.vector.reciprocal(sm[:, 1:2], sm[:, 0:1])
                        w2 = m2wp.tile([128, 128], bf16, tag="m2w", name="w2")
                        st[g]["w2"] = w2
                        kr = 128 if gs == G else 12 * gs
                        nc.gpsimd.tensor_scalar_mul(
                            w2[0:kr, :], m2psb[0:kr, :], sm[0:kr, 1:2])

                    def mix2T(g, gs):
                        # A2^T chunk: out[m', (rr,o)] = sum_rows
                        #   pg[row, m'] * w2[row, (rr,o)] -- m on partitions,
                        # written directly into the a2t (transposed) layout.
                        # K=128 (padded rows contribute exp(0)*0) -> FWL.
                        s = st.pop(g)
                        kr = 128 if gs == G else 12 * gs
                        a2 = psA2.tile([128, 8, 128], f32, tag="psA2",
                                       name="a2")
                        for c in range(8):
                            nc.tensor.matmul(
                                a2[:, c, :],
                                lhsT=s["pg"][0:kr, 128 * c:128 * c + 128],
                                rhs=s["w2"][0:kr, :],
                                start=True, stop=True,
                            )
                            if c == 3:
                                evac(a2t[:, g, 0:4, :], a2[:, 0:4, :], 512)
                        evac(a2t[:, g, 4:8, :], a2[:, 4:8, :], 512)

                    SKEW = 2
                    for i in range(len(steps) + SKEW):
                        if i < len(steps):
                            mix1(*steps[i])
                        if i >= SKEW:
                            mix2T(*steps[i - SKEW])
                        yield

                def av_thunks(bi):
                    """attention@V of block bi: 6 column-packed head-pair
                    chains accumulating into shared 1-bank psum tiles."""
                    n0, nb, chunks = blocks[bi]
                    a2t = a2t_t.pop(bi)
                    out = []
                    tiles = {}

                    def mk(j):
                        def emit():
                            grp, jj = divmod(j, 4)
                            npair = 4 if grp == 0 else 2
                            if jj == 0:
                                tiles[grp] = psAV.tile(
                                    [128, npair, 128], f32, tag="psAV",
                                    name="av", padded_shape=[128, 4, 128])
                            av = tiles[grp]
                            for half in range(2):
                                o = 2 * j + half
                                for (rs, g0, gc, gs) in chunks:
                                    for c in range(8):
                                        nc.tensor.matmul(
                                            av[64 * half:64 * half + 64, jj,
                                               rs:rs + gc * gs],
                                            lhsT=vsb[:, c, 64 * o:64 * o + 64],
                                            rhs=a2t[:, g0:g0 + gc, c,
                                                    o:o + 12 * (gs - 1) + 1:12],
                                            start=(c == 0), stop=(c == 7),
                                        )
                            if jj == npair - 1:
                                evac(attnT[:, 4 * grp:4 * grp + npair,
                                           n0:n0 + nb],
                                     av[:, :, 0:nb], npair * nb)
                        return emit

                    for j in range(6):
                        out.append(mk(j))
                    return out

                for it in range(nblk + 2):
                    avs = av_thunks(it - 2) if 2 <= it <= nblk + 1 else []
                    ss = s_thunks(it) if it < nblk else []
                    # 1. dense PE burst: AV chains woven with score pairs
                    si = 0
                    for t in avs:
                        t()
                        for _ in range(2):
                            if si < len(ss) - 1:
                                ss[si]()
                                si += 1
                    while si < len(ss) - 1:
                        ss[si]()
                        si += 1
                    # 2. interleave DMAs of block `it`
                    if ss:
                        ss[-1]()
                    # 3. mixes of block it-1
                    if 1 <= it <= nblk:
                        for _ in stage_M_gen(it - 1):
                            pass

            # ---------------- proj ----------------
            with tc.tile_pool(name="projps", bufs=3, space="PSUM") as pjp, \
                 tc.tile_pool(name="outp", bufs=3) as outp:
                wpsb = outp.tile([128, KC, C], bf16)
                nc.sync.dma_start(
                    wpsb, wprojt.rearrange("(kc p) c -> p kc c", p=128))
                od = out_d.rearrange("(ct p) n -> p ct n", p=128)
                for ct in range(KC):
                    for nh in range(2):
                        ps = pjp.tile([128, 512], f32, tag="pj")
                        for kc in range(KC):
                            nc.tensor.matmul(
                                ps,
                                lhsT=wpsb[:, kc, 128 * ct:128 * ct + 128],
                                rhs=attnT[:, kc, 512 * nh:512 * nh + 512],
                                start=(kc == 0), stop=(kc == KC - 1),
                            )
                        ob = outp.tile([128, 512], f32, tag="ob")
                        evac(ob, ps, 512)
                        nc.sync.dma_start(
                            od[:, ct, 512 * nh:512 * nh + 512], ob)

    nc.compile()
    return nc


def _mix_weights(conv_l_w, conv_w_w):
    """Host-built mix lhsT matrices, row map r = rr*12 + h.

    m1w[rr*12+h, rr*12+o] = SCALE * conv_l[o, h]   (lhsT for mix1)
    m2p[rr*12+h, rr*12+o] = conv_w[o, h]           (pattern, f32; scaled
        per-group on device by 1/softmax_sum per row; 128 cols, cols >= 120
        are zero so A2 psum rows [rows:128) are zeros)
    The gs=4 ragged group uses the leading [48, 48] / [48, :] slices.
    """
    m1 = np.zeros((120, 128), np.float32)
    m2 = np.zeros((128, 128), np.float32)
    for rr in range(G):
        for h in range(12):
            for o in range(12):
                m1[rr * 12 + h, rr * 12 + o] = SCALE * conv_l_w[o, h]
                m2[rr * 12 + h, rr * 12 + o] = conv_w_w[o, h]
    return m1.astype(BF16), m2.astype(np.float32)


def _perm():
    """Block-local query permutation: position n0+rr*gc+g holds query
    n0+g*gs+rr, so score-matmul output rows land pre-interleaved."""
    p = np.arange(N)
    for (n0, nb, chunks) in _block_layout():
        (rs, g0, gc, gs) = chunks[0]
        for rr in range(gs):
            for g in range(gc):
                p[n0 + rr * gc + g] = n0 + g * gs + rr
    return p


def _run(x, w_qkv, w_proj, b_proj, conv_l_w, conv_w_w, **spmd_kwargs):
    global _cached
    from concourse import bass_utils

    x = np.asarray(x, np.float32)
    w_qkv = np.asarray(w_qkv, np.float32)
    w_proj = np.asarray(w_proj, np.float32)
    b_proj = np.asarray(b_proj, np.float32)
    conv_l_w = np.asarray(conv_l_w, np.float32)
    conv_w_w = np.asarray(conv_w_w, np.float32)

    if _cached is None:
        _cached = _build_program()
    nc = _cached

    m1w, m2p = _mix_weights(conv_l_w, conv_w_w)
    wqkvt = np.ascontiguousarray(w_qkv.T).astype(BF16)
    wprojt = np.ascontiguousarray(w_proj.T).astype(BF16)

    perm = _perm()
    in_maps = []
    for b in range(B):
        in_maps.append({
            "xt": np.ascontiguousarray(x[b].T[:, perm]).astype(BF16),
            "wqkvt": wqkvt,
            "wprojt": wprojt,
            "m1w": m1w,
            "m2p": m2p,
        })

    res = bass_utils.run_bass_kernel_spmd(
        nc, in_maps, core_ids=list(range(B)), **spmd_kwargs)
    out = np.stack([res.results[b]["out"].T for b in range(B)])  # [B, N, C]
    return (out + b_proj[None, None, :]).astype(np.float32), res


def kernel(x, w_qkv, w_proj, b_proj, conv_l_w, conv_w_w):
    out, _ = _run(x, w_qkv, w_proj, b_proj, conv_l_w, conv_w_w)
    return out


# revision 18
# speedup vs baseline: 1.1539x; 1.0548x over previous
"""MiniAttention Trainium2 Bass kernel.

Problem: B=8, N=1024, C=768, H=12, D=64.
  qkv = x @ w_qkv.T ; q,k,v heads ; S = (q*SCALE) @ k.T per head
  A1 = conv_l-mix over heads ; P = softmax_m(A1) ; A2 = conv_w-mix over heads
  out = (A2 @ v per head) @ w_proj.T + b_proj
Sharding: pure batch-parallel, 1 batch element per NeuronCore (8 cores).

Per-core design (PE matmuls in bf16, f32 accumulation):
  - Host passes x^T, w_qkv^T, w_proj^T (transposed on host, bf16).
  - Scores per head h evac'd into S_all [block_row, (h, m)]; head-interleave
    into groups of G=10 queries with row map r = rr*12 + h via ONE SWDGE
    DMA per group (engines execute in order, and HWDGE dma_start costs
    ~600ns of Sync occupancy each, so both queue choice and instruction
    count matter).
  - conv_l (SCALE folded) as constant rr-block-diagonal lhsT; exp on ACT
    with accum_out giving softmax sums; normalization folded into the
    per-group conv_w lhsT (rows scaled by 1/sum, built on GpSimd).
  - A2 -> xbar DMA-transpose -> attention@V contracts m at K=128 with
    column-packed head pairs (full 128-partition PSUM tile).
  - All engines execute their instruction streams IN ORDER, so the block
    loop is software-pipelined by emission order: per iteration emit
    mixes(b-1) with a 3-half skew (mix2 lags mix1 so ACT exp latency is
    hidden) and the AV chains of block b-2 injected between mix steps,
    then scores(b) last. PSUM: scores+mix1 share one 4-buffer ring
    (tag "ps"), mix2 2 banks, AV 2 banks = 8 banks total.
  - PSUM->SBUF evacs are greedily balanced between ACT and DVE by
    accumulated estimated cost (exp is pinned to ACT, small ops to GpSimd).
"""

import numpy as np
import ml_dtypes

B, N, C, H = 8, 1024, 768, 12
D = C // H
SCALE = D ** -0.5
G = 10          # queries per mix group
NB = 120        # queries per block (12 groups)
NBLK = 8        # full blocks; last block is ragged: 6 groups of 10 + 1 of 4
BF16 = ml_dtypes.bfloat16

_cached = None


def _block_layout():
    """Returns list of blocks: (n0, nb, chunks) where chunks is a list of
    (row_start, g_start, g_count, g_size) describing the query groups."""
    blocks = []
    for b in range(NBLK):
        blocks.append((b * NB, NB, [(0, 0, 12, G)]))
    # ragged tail: n in [960, 1024) = 8 groups of 8
    blocks.append((960, 64, [(0, 0, 8, 8)]))
    return blocks


def _build_program():
    import concourse.tile as tile
    from concourse import bacc, mybir

    f32 = mybir.dt.float32
    bf16 = mybir.dt.bfloat16
    Exp = mybir.ActivationFunctionType.Exp

    nc = bacc.Bacc("TRN2", target_bir_lowering=False, debug=False)

    xt = nc.dram_tensor("xt", [C, N], bf16, kind="ExternalInput").ap()
    wqkvt = nc.dram_tensor("wqkvt", [C, 3 * C], bf16, kind="ExternalInput").ap()
    wprojt = nc.dram_tensor("wprojt", [C, C], bf16, kind="ExternalInput").ap()
    m1w_in = nc.dram_tensor("m1w", [12 * G, 128], bf16, kind="ExternalInput").ap()
    m2p_in = nc.dram_tensor("m2p", [128, 128], f32, kind="ExternalInput").ap()
    out_d = nc.dram_tensor("out", [C, N], f32, kind="ExternalOutput").ap()

    KC = C // 128  # 6 contraction chunks

    # greedy ACT/DVE balance by estimated occupancy (ns)
    _load = {"act": 0.0, "dve": 0.0}

    def evac(dst, src, n):
        """PSUM->SBUF copy of [rows, n]; pick the less-loaded engine."""
        if _load["act"] + 200 + n / 1.2 <= _load["dve"] + 200 + n / 0.96:
            _load["act"] += 200 + n / 1.2
            nc.scalar.copy(dst, src)
        else:
            _load["dve"] += 200 + n / 0.96
            nc.vector.tensor_copy(dst, src)

    with tile.TileContext(nc) as tc:
        with tc.tile_pool(name="const", bufs=1) as const, \
             tc.tile_pool(name="big", bufs=1) as big:

            m1wsb = const.tile([120, 128], bf16)
            nc.sync.dma_start(m1wsb, m1w_in)
            m2psb = const.tile([128, 128], f32)
            nc.sync.dma_start(m2psb, m2p_in)

            # persistent activations
            qksb = big.tile([128, 2 * KC, N], bf16)   # ct 0..5 = q, 6..11 = k
            vsb = big.tile([128, 8, C], bf16)         # [m%128, m//128, cout]
            attnT = big.tile([128, KC, N], bf16)      # [cout2%128, cout2//128, n]

            # ---------------- QKV ----------------
            with tc.tile_pool(name="xtp", bufs=1) as xtp, \
                 tc.tile_pool(name="qkvps", bufs=3, space="PSUM") as qkvps, \
                 tc.tile_pool(name="vps", bufs=2, space="PSUM") as vps:
                xtsb = xtp.tile([128, KC, N], bf16)
                wqsb = xtp.tile([128, KC, 3 * C], bf16)
                wqr = wqkvt.rearrange("(kc p) c -> p kc c", p=128)
                xtr = xt.rearrange("(kc p) n -> p kc n", p=128)
                # split loads so the first q matmuls start early
                nc.sync.dma_start(wqsb[:, :, 0:C], wqr[:, :, 0:C])
                nc.sync.dma_start(xtsb[:, :, 0:512], xtr[:, :, 0:512])
                nc.sync.dma_start(wqsb[:, :, C:2 * C], wqr[:, :, C:2 * C])
                nc.sync.dma_start(xtsb[:, :, 512:N], xtr[:, :, 512:N])
                nc.sync.dma_start(wqsb[:, :, 2 * C:3 * C], wqr[:, :, 2 * C:3 * C])

                # q, k: out[cout_tile, n]
                for ct in range(12):
                    for nh in range(2):
                        ps = qkvps.tile([128, 512], f32, tag="qkv")
                        for kc in range(KC):
                            nc.tensor.matmul(
                                ps,
                                lhsT=wqsb[:, kc, 128 * ct:128 * ct + 128],
                                rhs=xtsb[:, kc, 512 * nh:512 * nh + 512],
                                start=(kc == 0), stop=(kc == KC - 1),
                            )
                        evac(qksb[:, ct, 512 * nh:512 * nh + 512], ps, 512)

                # v: out[n_tile, cout]
                for nt in range(8):
                    ps = vps.tile([128, 768], f32, tag="vps")
                    for half, (c0, c1) in enumerate([(0, 512), (512, 768)]):
                        for kc in range(KC):
                            nc.tensor.matmul(
                                ps[:, c0:c1],
                                lhsT=xtsb[:, kc, 128 * nt:128 * nt + 128],
                                rhs=wqsb[:, kc, 2 * C + c0:2 * C + c1],
                                start=(kc == 0), stop=(kc == KC - 1),
                            )
                    evac(vsb[:, nt, :], ps, 768)

            # ------- attention: software-pipelined over blocks -------
            # Per iteration (emission order == execution order per engine):
            #   1. dense PE burst: AV chains of block b-2 woven with score
            #      matmul pairs of block b (keeps the PE busy enough for the
            #      HAM clock-gate to hold 2.4 GHz; score evacs drain on
            #      ACT/DVE underneath),
            #   2. interleave DMAs of block b (sync/gpsimd alternating),
            #   3. mix steps of block b-1 (mix2T lags mix1 by SKEW to hide
            #      the ACT exp latency).
            # Mix matmuls carry 128 weight columns so the compiler enables
            # FWL (4x faster LDWEIGHTS): m1w is column-padded with zeros, so
            # a1 rows [rows:128) are written as zeros, exp turns them into
            # finite 1s, and m2p's zero rows [120:128) zero them out of A2.
            blocks = _block_layout()
            nblk = len(blocks)

            with tc.tile_pool(name="sallp", bufs=1) as sallp, \
                 tc.tile_pool(name="sintp", bufs=2) as sintp, \
                 tc.tile_pool(name="pintp", bufs=5) as pintp, \
                 tc.tile_pool(name="a2tp", bufs=2) as a2tp, \
                 tc.tile_pool(name="smp", bufs=8) as smp, \
                 tc.tile_pool(name="m2wp", bufs=6) as m2wp, \
                 tc.tile_pool(name="psmix", bufs=2, space="PSUM") as psmix, \
                 tc.tile_pool(name="psA2", bufs=3, space="PSUM") as psA2, \
                 tc.tile_pool(name="psAV", bufs=1, space="PSUM") as psAV:

                sint_t = {}
                a2t_t = {}
                _dq = [0]

                def s_thunks(bi):
                    """Scores of block bi: 12 per-head thunks (2 MMs into a
                    2-bank psum tile + one evac), then the interleave DMAs."""
                    n0, nb, chunks = blocks[bi]
                    sall = sallp.tile([128, 12, N], bf16, tag="sall",
                                      name="sall")
                    out = []

                    (rs0, g00, gc, gs) = chunks[0]

                    def mk(h):
                        def emit():
                            base = 64 * (h % 2)
                            ps = psmix.tile([128, 1024], f32, tag="ps",
                                            name="ps")
                            # x columns are block-locally permuted on the
                            # host so that psum row rr*gc+g holds query
                            # n0+g*gs+rr: the interleave below then reads a
                            # stride-gc partition slice (spreads across all
                            # SDMA engine ports), and the permutation
                            # cancels at the AV stage.
                            for mh in range(2):
                                nc.tensor.matmul(
                                    ps[0:nb, 512 * mh:512 * mh + 512],
                                    lhsT=qksb[base:base + 64, h // 2,
                                              n0:n0 + nb],
                                    rhs=qksb[base:base + 64, 6 + h // 2,
                                             512 * mh:512 * mh + 512],
                                    start=True, stop=True,
                                )
                            evac(sall[0:nb, h, :], ps[0:nb, :], 1024)
                        return emit

                    for h in range(12):
                        out.append(mk(h))

                    def interleave():
                        # sint[rr*12+h, g, m] = sall[rr*gc+g, h, m]
                        sint = sintp.tile([128, 12, N], bf16, tag="sint",
                                          name="sint")
                        sint_t[bi] = sint
                        for g in range(gc):
                            eng = nc.sync if _dq[0] % 2 == 0 else nc.gpsimd
                            _dq[0] += 1
                            eng.dma_start(
                                out=sint[0:12 * gs, g, :],
                                in_=sall[g:gc * gs:gc, :, :],
                            )
                    out.append(interleave)
                    return out

                def stage_M_gen(bi):
                    """Mixes of block bi at group steps; mix2 (transposed,
                    straight into a2t layout) lags mix1 by SKEW steps."""
                    n0, nb, chunks = blocks[bi]
                    sint = sint_t.pop(bi)
                    a2t = a2tp.tile([128, 12, 8, 128], bf16, tag="a2t",
                                    name="a2t")
                    a2t_t[bi] = a2t
                    steps = []
                    for (rs, g0, gc, gs) in chunks:
                        for g in range(g0, g0 + gc):
                            steps.append((g, gs))
                    st = {}

                    def mix1(g, gs):
                        rows = 12 * gs
                        pg = pintp.tile([128, N], bf16, tag="pint", name="pg")
                        sm = smp.tile([128, 2], f32, tag="sm", name="sm")
                        st[g] = {"pg": pg, "sm": sm, "w2": None, "gs": gs}
                        a1 = psmix.tile([128, 1024], f32, tag="ps", name="a1")
                        for mh in range(2):
                            nc.tensor.matmul(
                                a1[:, 512 * mh:512 * mh + 512],
                                lhsT=m1wsb[0:rows, 0:128],
                                rhs=sint[0:rows, g, 512 * mh:512 * mh + 512],
                                start=True, stop=True,
                            )
                        _load["act"] += 1050
                        nc.scalar.activation(
                            pg, a1, Exp,
                            accum_out=sm[:, 0:1],
                        )
                        _load["dve"] += 150
                        nc.vector.reciprocal(sm[:, 1:2], sm[:, 0:1])
                        w2 = m2wp.tile([128, 128], bf16, tag="m2w", name="w2")
                        st[g]["w2"] = w2
                        kr = 128 if gs == G else 12 * gs
                        nc.gpsimd.tensor_scalar_mul(
                            w2[0:kr, :], m2psb[0:kr, :], sm[0:kr, 1:2])

                    def mix2T(g, gs):
                        # A2^T chunk: out[m', (rr,o)] = sum_rows
                        #   pg[row, m'] * w2[row, (rr,o)] -- m on partitions,
                        # written directly into the a2t (transposed) layout.
                        # K=128 (padded rows contribute exp(0)*0) -> FWL.
                        s = st.pop(g)
                        kr = 128 if gs == G else 12 * gs
                        for hf in range(2):
                            a2 = psA2.tile([128, 4, 128], f32, tag="psA2",
                                           name="a2")
                            for cc in range(4):
                                c = 4 * hf + cc
                                nc.tensor.matmul(
                                    a2[:, cc, :],
                                    lhsT=s["pg"][0:kr, 128 * c:128 * c + 128],
                                    rhs=s["w2"][0:kr, :],
                                    start=True, stop=True,
                                )
                            evac(a2t[:, g, 4 * hf:4 * hf + 4, :], a2, 512)

                    SKEW = 3
                    for i in range(len(steps) + SKEW):
                        if i < len(steps):
                            mix1(*steps[i])
                        if i >= SKEW:
                            mix2T(*steps[i - SKEW])
                        yield

                def av_thunks(bi):
                    """attention@V of block bi: 6 column-packed head-pair
                    chains accumulating into shared 1-bank psum tiles."""
                    n0, nb, chunks = blocks[bi]
                    a2t = a2t_t.pop(bi)
                    out = []
                    tiles = {}

                    def mk(j):
                        def emit():
                            grp, jj = divmod(j, 4)
                            npair = 4 if grp == 0 else 2
                            if jj == 0:
                                tiles[grp] = psAV.tile(
                                    [128, npair, 128], f32, tag="psAV",
                                    name="av", padded_shape=[128, 4, 128])
                            av = tiles[grp]
                            for half in range(2):
                                o = 2 * j + half
                                for (rs, g0, gc, gs) in chunks:
                                    for c in range(8):
                                        nc.tensor.matmul(
                                            av[64 * half:64 * half + 64, jj,
                                               rs:rs + gc * gs],
                                            lhsT=vsb[:, c, 64 * o:64 * o + 64],
                                            rhs=a2t[:, g0:g0 + gc, c,
                                                    o:o + 12 * (gs - 1) + 1:12],
                                            start=(c == 0), stop=(c == 7),
                                        )
                            if jj == npair - 1:
                                evac(attnT[:, 4 * grp:4 * grp + npair,
                                           n0:n0 + nb],
                                     av[:, :, 0:nb], npair * nb)
                        return emit

                    for j in range(6):
                        out.append(mk(j))
                    return out

                for it in range(nblk + 2):
                    avs = av_thunks(it - 2) if 2 <= it <= nblk + 1 else []
                    ss = s_thunks(it) if it < nblk else []
                    # 1. dense PE burst: AV chains woven with score pairs
                    si = 0
                    for t in avs:
                        t()
                        for _ in range(2):
                            if si < len(ss) - 1:
                                ss[si]()
                                si += 1
                    while si < len(ss) - 1:
                        ss[si]()
                        si += 1
                    # 2. interleave DMAs of block `it`
                    if ss:
                        ss[-1]()
                    # 3. mixes of block it-1
                    if 1 <= it <= nblk:
                        for _ in stage_M_gen(it - 1):
                            pass

            # ---------------- proj ----------------
            with tc.tile_pool(name="projps", bufs=3, space="PSUM") as pjp, \
                 tc.tile_pool(name="outp", bufs=3) as outp:
                wpsb = outp.tile([128, KC, C], bf16)
                nc.sync.dma_start(
                    wpsb, wprojt.rearrange("(kc p) c -> p kc c", p=128))
                od = out_d.rearrange("(ct p) n -> p ct n", p=128)
                for ct in range(KC):
                    for nh in range(2):
                        ps = pjp.tile([128, 512], f32, tag="pj")
                        for kc in range(KC):
                            nc.tensor.matmul(
                                ps,
                                lhsT=wpsb[:, kc, 128 * ct:128 * ct + 128],
                                rhs=attnT[:, kc, 512 * nh:512 * nh + 512],
                                start=(kc == 0), stop=(kc == KC - 1),
                            )
                        ob = outp.tile([128, 512], f32, tag="ob")
                        evac(ob, ps, 512)
                        nc.sync.dma_start(
                            od[:, ct, 512 * nh:512 * nh + 512], ob)

    nc.compile()
    return nc


def _mix_weights(conv_l_w, conv_w_w):
    """Host-built mix lhsT matrices, row map r = rr*12 + h.

    m1w[rr*12+h, rr*12+o] = SCALE * conv_l[o, h]   (lhsT for mix1)
    m2p[rr*12+h, rr*12+o] = conv_w[o, h]           (pattern, f32; scaled
        per-group on device by 1/softmax_sum per row; 128 cols, cols >= 120
        are zero so A2 psum rows [rows:128) are zeros)
    The gs=4 ragged group uses the leading [48, 48] / [48, :] slices.
    """
    m1 = np.zeros((120, 128), np.float32)
    m2 = np.zeros((128, 128), np.float32)
    for rr in range(G):
        for h in range(12):
            for o in range(12):
                m1[rr * 12 + h, rr * 12 + o] = SCALE * conv_l_w[o, h]
                m2[rr * 12 + h, rr * 12 + o] = conv_w_w[o, h]
    return m1.astype(BF16), m2.astype(np.float32)


def _perm():
    """Block-local query permutation: position n0+rr*gc+g holds query
    n0+g*gs+rr, so score-matmul output rows land pre-interleaved."""
    p = np.arange(N)
    for (n0, nb, chunks) in _block_layout():
        (rs, g0, gc, gs) = chunks[0]
        for rr in range(gs):
            for g in range(gc):
                p[n0 + rr * gc + g] = n0 + g * gs + rr
    return p


def _run(x, w_qkv, w_proj, b_proj, conv_l_w, conv_w_w, **spmd_kwargs):
    global _cached
    from concourse import bass_utils

    x = np.asarray(x, np.float32)
    w_qkv = np.asarray(w_qkv, np.float32)
    w_proj = np.asarray(w_proj, np.float32)
    b_proj = np.asarray(b_proj, np.float32)
    conv_l_w = np.asarray(conv_l_w, np.float32)
    conv_w_w = np.asarray(conv_w_w, np.float32)

    if _cached is None:
        _cached = _build_program()
    nc = _cached

    m1w, m2p = _mix_weights(conv_l_w, conv_w_w)
    wqkvt = np.ascontiguousarray(w_qkv.T).astype(BF16)
    wprojt = np.ascontiguousarray(w_proj.T).astype(BF16)

    perm = _perm()
    in_maps = []
    for b in range(B):
        in_maps.append({
            "xt": np.ascontiguousarray(x[b].T[:, perm]).astype(BF16),
            "wqkvt": wqkvt,
            "wprojt": wprojt,
            "m1w": m1w,
            "m2p": m2p,
        })

    res = bass_utils.run_bass_kernel_spmd(
        nc, in_maps, core_ids=list(range(B)), **spmd_kwargs)
    out = np.stack([res.results[b]["out"].T for b in range(B)])  # [B, N, C]
    return (out + b_proj[None, None, :]).astype(np.float32), res


def kernel(x, w_qkv, w_proj, b_proj, conv_l_w, conv_w_w):
    out, _ = _run(x, w_qkv, w_proj, b_proj, conv_l_w, conv_w_w)
    return out


# revision 19
# speedup vs baseline: 1.3077x; 1.1333x over previous
"""MiniAttention Trainium2 Bass kernel.

Problem: B=8, N=1024, C=768, H=12, D=64.
  qkv = x @ w_qkv.T ; q,k,v heads ; S = (q*SCALE) @ k.T per head
  A1 = conv_l-mix over heads ; P = softmax_m(A1) ; A2 = conv_w-mix over heads
  out = (A2 @ v per head) @ w_proj.T + b_proj
Sharding: pure batch-parallel, 1 batch element per NeuronCore (8 cores).

Per-core design (PE matmuls in bf16, f32 accumulation):
  - Host passes x^T, w_qkv^T, w_proj^T (transposed on host, bf16).
  - Scores per head h evac'd into S_all [block_row, (h, m)]; head-interleave
    into groups of G=10 queries with row map r = rr*12 + h via ONE SWDGE
    DMA per group (engines execute in order, and HWDGE dma_start costs
    ~600ns of Sync occupancy each, so both queue choice and instruction
    count matter).
  - conv_l (SCALE folded) as constant rr-block-diagonal lhsT; exp on ACT
    with accum_out giving softmax sums; normalization folded into the
    per-group conv_w lhsT (rows scaled by 1/sum, built on GpSimd).
  - A2 -> xbar DMA-transpose -> attention@V contracts m at K=128 with
    column-packed head pairs (full 128-partition PSUM tile).
  - All engines execute their instruction streams IN ORDER, so the block
    loop is software-pipelined by emission order: per iteration emit
    mixes(b-1) with a 3-half skew (mix2 lags mix1 so ACT exp latency is
    hidden) and the AV chains of block b-2 injected between mix steps,
    then scores(b) last. PSUM: scores+mix1 share one 4-buffer ring
    (tag "ps"), mix2 2 banks, AV 2 banks = 8 banks total.
  - PSUM->SBUF evacs are greedily balanced between ACT and DVE by
    accumulated estimated cost (exp is pinned to ACT, small ops to GpSimd).
"""

import numpy as np
import ml_dtypes

B, N, C, H = 8, 1024, 768, 12
D = C // H
SCALE = D ** -0.5
G = 10          # queries per mix group
NB = 120        # queries per block (12 groups)
NBLK = 8        # full blocks; last block is ragged: 6 groups of 10 + 1 of 4
BF16 = ml_dtypes.bfloat16

_cached = None


def _block_layout():
    """Returns list of blocks: (n0, nb, chunks) where chunks is a list of
    (row_start, g_start, g_count, g_size) describing the query groups."""
    blocks = []
    for b in range(NBLK):
        blocks.append((b * NB, NB, [(0, 0, 12, G)]))
    # ragged tail: n in [960, 1024) = 8 groups of 8
    blocks.append((960, 64, [(0, 0, 8, 8)]))
    return blocks


def _build_program():
    import concourse.tile as tile
    from concourse import bacc, mybir

    f32 = mybir.dt.float32
    bf16 = mybir.dt.bfloat16
    Exp = mybir.ActivationFunctionType.Exp

    nc = bacc.Bacc("TRN2", target_bir_lowering=False, debug=False)

    xt = nc.dram_tensor("xt", [C, N], bf16, kind="ExternalInput").ap()
    wqkvt = nc.dram_tensor("wqkvt", [C, 3 * C], bf16, kind="ExternalInput").ap()
    wprojt = nc.dram_tensor("wprojt", [C, C], bf16, kind="ExternalInput").ap()
    m1w_in = nc.dram_tensor("m1w", [12 * G, 128], bf16, kind="ExternalInput").ap()
    m2p_in = nc.dram_tensor("m2p", [128, 128], f32, kind="ExternalInput").ap()
    out_d = nc.dram_tensor("out", [C, N], f32, kind="ExternalOutput").ap()

    KC = C // 128  # 6 contraction chunks

    # greedy ACT/DVE balance by estimated occupancy (ns)
    _load = {"act": 0.0, "dve": 0.0}

    def evac(dst, src, n):
        """PSUM->SBUF copy of [rows, n]; pick the less-loaded engine."""
        if _load["act"] + 200 + n / 1.2 <= _load["dve"] + 200 + n / 0.96:
            _load["act"] += 200 + n / 1.2
            nc.scalar.copy(dst, src)
        else:
            _load["dve"] += 200 + n / 0.96
            nc.vector.tensor_copy(dst, src)

    with tile.TileContext(nc) as tc:
        with tc.tile_pool(name="const", bufs=1) as const, \
             tc.tile_pool(name="big", bufs=1) as big:

            m1wsb = const.tile([120, 128], bf16)
            nc.sync.dma_start(m1wsb, m1w_in)
            m2psb = const.tile([128, 128], f32)
            nc.sync.dma_start(m2psb, m2p_in)

            # persistent activations
            qksb = big.tile([128, 2 * KC, N], bf16)   # ct 0..5 = q, 6..11 = k
            vsb = big.tile([128, 8, C], bf16)         # [m%128, m//128, cout]
            attnT = big.tile([128, KC, N], bf16)      # [cout2%128, cout2//128, n]

            # ---------------- QKV ----------------
            with tc.tile_pool(name="xtp", bufs=1) as xtp, \
                 tc.tile_pool(name="qkvps", bufs=3, space="PSUM") as qkvps, \
                 tc.tile_pool(name="vps", bufs=2, space="PSUM") as vps:
                xtsb = xtp.tile([128, KC, N], bf16)
                wqsb = xtp.tile([128, KC, 3 * C], bf16)
                wqr = wqkvt.rearrange("(kc p) c -> p kc c", p=128)
                xtr = xt.rearrange("(kc p) n -> p kc n", p=128)
                # split loads so the first q matmuls start early
                nc.sync.dma_start(wqsb[:, :, 0:C], wqr[:, :, 0:C])
                nc.sync.dma_start(xtsb[:, :, 0:512], xtr[:, :, 0:512])
                nc.sync.dma_start(wqsb[:, :, C:2 * C], wqr[:, :, C:2 * C])
                nc.sync.dma_start(xtsb[:, :, 512:N], xtr[:, :, 512:N])
                nc.sync.dma_start(wqsb[:, :, 2 * C:3 * C], wqr[:, :, 2 * C:3 * C])

                # q, k: out[cout_tile, n]
                for ct in range(12):
                    for nh in range(2):
                        ps = qkvps.tile([128, 512], f32, tag="qkv")
                        for kc in range(KC):
                            nc.tensor.matmul(
                                ps,
                                lhsT=wqsb[:, kc, 128 * ct:128 * ct + 128],
                                rhs=xtsb[:, kc, 512 * nh:512 * nh + 512],
                                start=(kc == 0), stop=(kc == KC - 1),
                            )
                        evac(qksb[:, ct, 512 * nh:512 * nh + 512], ps, 512)

                # v: out[n_tile, cout]
                for nt in range(8):
                    ps = vps.tile([128, 768], f32, tag="vps")
                    for half, (c0, c1) in enumerate([(0, 512), (512, 768)]):
                        for kc in range(KC):
                            nc.tensor.matmul(
                                ps[:, c0:c1],
                                lhsT=xtsb[:, kc, 128 * nt:128 * nt + 128],
                                rhs=wqsb[:, kc, 2 * C + c0:2 * C + c1],
                                start=(kc == 0), stop=(kc == KC - 1),
                            )
                    evac(vsb[:, nt, :], ps, 768)

            # ------- attention: software-pipelined over blocks -------
            # Per iteration (emission order == execution order per engine):
            #   1. dense PE burst: AV chains of block b-2 woven with score
            #      matmul pairs of block b (keeps the PE busy enough for the
            #      HAM clock-gate to hold 2.4 GHz; score evacs drain on
            #      ACT/DVE underneath),
            #   2. interleave DMAs of block b (sync/gpsimd alternating),
            #   3. mix steps of block b-1 (mix2T lags mix1 by SKEW to hide
            #      the ACT exp latency).
            # Mix matmuls carry 128 weight columns so the compiler enables
            # FWL (4x faster LDWEIGHTS): m1w is column-padded with zeros, so
            # a1 rows [rows:128) are written as zeros, exp turns them into
            # finite 1s, and m2p's zero rows [120:128) zero them out of A2.
            blocks = _block_layout()
            nblk = len(blocks)

            with tc.tile_pool(name="sallp", bufs=1) as sallp, \
                 tc.tile_pool(name="sintp", bufs=2) as sintp, \
                 tc.tile_pool(name="pintp", bufs=5) as pintp, \
                 tc.tile_pool(name="a2tp", bufs=2) as a2tp, \
                 tc.tile_pool(name="smp", bufs=8) as smp, \
                 tc.tile_pool(name="m2wp", bufs=6) as m2wp, \
                 tc.tile_pool(name="psmix", bufs=2, space="PSUM") as psmix, \
                 tc.tile_pool(name="psA2", bufs=3, space="PSUM") as psA2, \
                 tc.tile_pool(name="psAV", bufs=1, space="PSUM") as psAV:

                sint_t = {}
                a2t_t = {}
                _dq = [0]

                def s_thunks(bi):
                    """Scores of block bi: 12 per-head thunks (2 MMs into a
                    2-bank psum tile + one evac), then the interleave DMAs."""
                    n0, nb, chunks = blocks[bi]
                    sall = sallp.tile([128, 12, N], bf16, tag="sall",
                                      name="sall")
                    out = []

                    (rs0, g00, gc, gs) = chunks[0]

                    def mk(h):
                        def emit():
                            base = 64 * (h % 2)
                            ps = psmix.tile([128, 1024], f32, tag="ps",
                                            name="ps")
                            # x columns are block-locally permuted on the
                            # host so that psum row rr*gc+g holds query
                            # n0+g*gs+rr: the interleave below then reads a
                            # stride-gc partition slice (spreads across all
                            # SDMA engine ports), and the permutation
                            # cancels at the AV stage.
                            for mh in range(2):
                                nc.tensor.matmul(
                                    ps[0:nb, 512 * mh:512 * mh + 512],
                                    lhsT=qksb[base:base + 64, h // 2,
                                              n0:n0 + nb],
                                    rhs=qksb[base:base + 64, 6 + h // 2,
                                             512 * mh:512 * mh + 512],
                                    start=True, stop=True,
                                )
                            evac(sall[0:nb, h, :], ps[0:nb, :], 1024)
                        return emit

                    for h in range(12):
                        out.append(mk(h))

                    def interleave():
                        # sint[rr*12+h, g, m] = sall[rr*gc+g, h, m]
                        sint = sintp.tile([128, 12, N], bf16, tag="sint",
                                          name="sint")
                        sint_t[bi] = sint
                        for g in range(gc):
                            eng = nc.sync if _dq[0] % 2 == 0 else nc.gpsimd
                            _dq[0] += 1
                            eng.dma_start(
                                out=sint[0:12 * gs, g, :],
                                in_=sall[g:gc * gs:gc, :, :],
                            )
                    out.append(interleave)
                    return out

                def stage_M_gen(bi):
                    """Mixes of block bi at group steps; mix2 (transposed,
                    straight into a2t layout) lags mix1 by SKEW steps."""
                    n0, nb, chunks = blocks[bi]
                    sint = sint_t.pop(bi)
                    a2t = a2tp.tile([128, 12, 8, 128], bf16, tag="a2t",
                                    name="a2t")
                    a2t_t[bi] = a2t
                    steps = []
                    for (rs, g0, gc, gs) in chunks:
                        for g in range(g0, g0 + gc):
                            steps.append((g, gs))
                    st = {}

                    def mix1(g, gs):
                        rows = 12 * gs
                        pg = pintp.tile([128, N], bf16, tag="pint", name="pg")
                        sm = smp.tile([128, 2], f32, tag="sm", name="sm")
                        st[g] = {"pg": pg, "sm": sm, "w2": None, "gs": gs}
                        a1 = psmix.tile([128, 1024], f32, tag="ps", name="a1")
                        for mh in range(2):
                            nc.tensor.matmul(
                                a1[:, 512 * mh:512 * mh + 512],
                                lhsT=m1wsb[0:rows, 0:128],
                                rhs=sint[0:rows, g, 512 * mh:512 * mh + 512],
                                start=True, stop=True,
                            )
                        _load["act"] += 1050
                        nc.scalar.activation(
                            pg, a1, Exp,
                            accum_out=sm[:, 0:1],
                        )
                        _load["dve"] += 400
                        nc.vector.reciprocal(sm[:, 1:2], sm[:, 0:1])
                        w2 = m2wp.tile([128, 128], bf16, tag="m2w", name="w2")
                        st[g]["w2"] = w2
                        kr = 128 if gs == G else 12 * gs
                        # on DVE right behind the reciprocal: gpsimd tensor
                        # ops cost ~2us of Q7 dispatch and sat in the
                        # exp->w2->mix2T critical chain
                        nc.vector.tensor_scalar_mul(
                            w2[0:kr, :], m2psb[0:kr, :], sm[0:kr, 1:2])

                    def mix2T(g, gs):
                        # A2^T chunk: out[m', (rr,o)] = sum_rows
                        #   pg[row, m'] * w2[row, (rr,o)] -- m on partitions,
                        # written directly into the a2t (transposed) layout.
                        # K=128 (padded rows contribute exp(0)*0) -> FWL.
                        s = st.pop(g)
                        kr = 128 if gs == G else 12 * gs
                        for hf in range(2):
                            a2 = psA2.tile([128, 4, 128], f32, tag="psA2",
                                           name="a2")
                            for cc in range(4):
                                c = 4 * hf + cc
                                nc.tensor.matmul(
                                    a2[:, cc, :],
                                    lhsT=s["pg"][0:kr, 128 * c:128 * c + 128],
                                    rhs=s["w2"][0:kr, :],
                                    start=True, stop=True,
                                )
                            evac(a2t[:, g, 4 * hf:4 * hf + 4, :], a2, 512)

                    SKEW = 3
                    for i in range(len(steps) + SKEW):
                        if i < len(steps):
                            mix1(*steps[i])
                        if i >= SKEW:
                            mix2T(*steps[i - SKEW])
                        yield

                def av_thunks(bi):
                    """attention@V of block bi: 6 column-packed head-pair
                    chains accumulating into shared 1-bank psum tiles."""
                    n0, nb, chunks = blocks[bi]
                    a2t = a2t_t.pop(bi)
                    out = []
                    tiles = {}

                    def mk(j):
                        def emit():
                            grp, jj = divmod(j, 4)
                            npair = 4 if grp == 0 else 2
                            if jj == 0:
                                tiles[grp] = psAV.tile(
                                    [128, npair, 128], f32, tag="psAV",
                                    name="av", padded_shape=[128, 4, 128])
                            av = tiles[grp]
                            for half in range(2):
                                o = 2 * j + half
                                for (rs, g0, gc, gs) in chunks:
                                    for c in range(8):
                                        nc.tensor.matmul(
                                            av[64 * half:64 * half + 64, jj,
                                               rs:rs + gc * gs],
                                            lhsT=vsb[:, c, 64 * o:64 * o + 64],
                                            rhs=a2t[:, g0:g0 + gc, c,
                                                    o:o + 12 * (gs - 1) + 1:12],
                                            start=(c == 0), stop=(c == 7),
                                        )
                            if jj == npair - 1:
                                evac(attnT[:, 4 * grp:4 * grp + npair,
                                           n0:n0 + nb],
                                     av[:, :, 0:nb], npair * nb)
                        return emit

                    for j in range(6):
                        out.append(mk(j))
                    return out

                for it in range(nblk + 2):
                    avs = av_thunks(it - 2) if 2 <= it <= nblk + 1 else []
                    ss = s_thunks(it) if it < nblk else []
                    # 1. dense PE burst: AV chains woven with score pairs
                    si = 0
                    for t in avs:
                        t()
                        for _ in range(2):
                            if si < len(ss) - 1:
                                ss[si]()
                                si += 1
                    while si < len(ss) - 1:
                        ss[si]()
                        si += 1
                    # 2. interleave DMAs of block `it`
                    if ss:
                        ss[-1]()
                    # 3. mixes of block it-1
                    if 1 <= it <= nblk:
                        for _ in stage_M_gen(it - 1):
                            pass

            # ---------------- proj ----------------
            with tc.tile_pool(name="projps", bufs=3, space="PSUM") as pjp, \
                 tc.tile_pool(name="outp", bufs=3) as outp:
                wpsb = outp.tile([128, KC, C], bf16)
                nc.sync.dma_start(
                    wpsb, wprojt.rearrange("(kc p) c -> p kc c", p=128))
                od = out_d.rearrange("(ct p) n -> p ct n", p=128)
                for ct in range(KC):
                    for nh in range(2):
                        ps = pjp.tile([128, 512], f32, tag="pj")
                        for kc in range(KC):
                            nc.tensor.matmul(
                                ps,
                                lhsT=wpsb[:, kc, 128 * ct:128 * ct + 128],
                                rhs=attnT[:, kc, 512 * nh:512 * nh + 512],
                                start=(kc == 0), stop=(kc == KC - 1),
                            )
                        ob = outp.tile([128, 512], f32, tag="ob")
                        evac(ob, ps, 512)
                        nc.sync.dma_start(
                            od[:, ct, 512 * nh:512 * nh + 512], ob)

    nc.compile()
    return nc


def _mix_weights(conv_l_w, conv_w_w):
    """Host-built mix lhsT matrices, row map r = rr*12 + h.

    m1w[rr*12+h, rr*12+o] = SCALE * conv_l[o, h]   (lhsT for mix1)
    m2p[rr*12+h, rr*12+o] = conv_w[o, h]           (pattern, f32; scaled
        per-group on device by 1/softmax_sum per row; 128 cols, cols >= 120
        are zero so A2 psum rows [rows:128) are zeros)
    The gs=4 ragged group uses the leading [48, 48] / [48, :] slices.
    """
    m1 = np.zeros((120, 128), np.float32)
    m2 = np.zeros((128, 128), np.float32)
    for rr in range(G):
        for h in range(12):
            for o in range(12):
                m1[rr * 12 + h, rr * 12 + o] = SCALE * conv_l_w[o, h]
                m2[rr * 12 + h, rr * 12 + o] = conv_w_w[o, h]
    return m1.astype(BF16), m2.astype(np.float32)


def _perm():
    """Block-local query permutation: position n0+rr*gc+g holds query
    n0+g*gs+rr, so score-matmul output rows land pre-interleaved."""
    p = np.arange(N)
    for (n0, nb, chunks) in _block_layout():
        (rs, g0, gc, gs) = chunks[0]
        for rr in range(gs):
            for g in range(gc):
                p[n0 + rr * gc + g] = n0 + g * gs + rr
    return p


def _run(x, w_qkv, w_proj, b_proj, conv_l_w, conv_w_w, **spmd_kwargs):
    global _cached
    from concourse import bass_utils

    x = np.asarray(x, np.float32)
    w_qkv = np.asarray(w_qkv, np.float32)
    w_proj = np.asarray(w_proj, np.float32)
    b_proj = np.asarray(b_proj, np.float32)
    conv_l_w = np.asarray(conv_l_w, np.float32)
    conv_w_w = np.asarray(conv_w_w, np.float32)

    if _cached is None:
        _cached = _build_program()
    nc = _cached

    m1w, m2p = _mix_weights(conv_l_w, conv_w_w)
    wqkvt = np.ascontiguousarray(w_qkv.T).astype(BF16)
    wprojt = np.ascontiguousarray(w_proj.T).astype(BF16)

    perm = _perm()
    in_maps = []
    for b in range(B):
        in_maps.append({
            "xt": np.ascontiguousarray(x[b].T[:, perm]).astype(BF16),
            "wqkvt": wqkvt,
            "wprojt": wprojt,
            "m1w": m1w,
            "m2p": m2p,
        })

    res = bass_utils.run_bass_kernel_spmd(
        nc, in_maps, core_ids=list(range(B)), **spmd_kwargs)
    out = np.stack([res.results[b]["out"].T for b in range(B)])  # [B, N, C]
    return (out + b_proj[None, None, :]).astype(np.float32), res


def kernel(x, w_qkv, w_proj, b_proj, conv_l_w, conv_w_w):
    out, _ = _run(x, w_qkv, w_proj, b_proj, conv_l_w, conv_w_w)
    return out


# revision 20
# speedup vs baseline: 1.3502x; 1.0325x over previous
"""MiniAttention Trainium2 Bass kernel.

Problem: B=8, N=1024, C=768, H=12, D=64.
  qkv = x @ w_qkv.T ; q,k,v heads ; S = (q*SCALE) @ k.T per head
  A1 = conv_l-mix over heads ; P = softmax_m(A1) ; A2 = conv_w-mix over heads
  out = (A2 @ v per head) @ w_proj.T + b_proj
Sharding: pure batch-parallel, 1 batch element per NeuronCore (8 cores).

Per-core design (PE matmuls in bf16, f32 accumulation):
  - Host passes x^T, w_qkv^T, w_proj^T (transposed on host, bf16).
  - Scores per head h evac'd into S_all [block_row, (h, m)]; head-interleave
    into groups of G=10 queries with row map r = rr*12 + h via ONE SWDGE
    DMA per group (engines execute in order, and HWDGE dma_start costs
    ~600ns of Sync occupancy each, so both queue choice and instruction
    count matter).
  - conv_l (SCALE folded) as constant rr-block-diagonal lhsT; exp on ACT
    with accum_out giving softmax sums; normalization folded into the
    per-group conv_w lhsT (rows scaled by 1/sum, built on GpSimd).
  - A2 -> xbar DMA-transpose -> attention@V contracts m at K=128 with
    column-packed head pairs (full 128-partition PSUM tile).
  - All engines execute their instruction streams IN ORDER, so the block
    loop is software-pipelined by emission order: per iteration emit
    mixes(b-1) with a 3-half skew (mix2 lags mix1 so ACT exp latency is
    hidden) and the AV chains of block b-2 injected between mix steps,
    then scores(b) last. PSUM: scores+mix1 share one 4-buffer ring
    (tag "ps"), mix2 2 banks, AV 2 banks = 8 banks total.
  - PSUM->SBUF evacs are greedily balanced between ACT and DVE by
    accumulated estimated cost (exp is pinned to ACT, small ops to GpSimd).
"""

import numpy as np
import ml_dtypes

B, N, C, H = 8, 1024, 768, 12
D = C // H
SCALE = D ** -0.5
G = 10          # queries per mix group
NB = 120        # queries per block (12 groups)
NBLK = 8        # full blocks; last block is ragged: 6 groups of 10 + 1 of 4
BF16 = ml_dtypes.bfloat16

_cached = None


def _block_layout():
    """Returns list of blocks: (n0, nb, chunks) where chunks is a list of
    (row_start, g_start, g_count, g_size) describing the query groups."""
    blocks = []
    for b in range(NBLK):
        blocks.append((b * NB, NB, [(0, 0, 12, G)]))
    # ragged tail: n in [960, 1024) = 8 groups of 8
    blocks.append((960, 64, [(0, 0, 8, 8)]))
    return blocks


def _build_program():
    import concourse.tile as tile
    from concourse import bacc, mybir

    f32 = mybir.dt.float32
    bf16 = mybir.dt.bfloat16
    Exp = mybir.ActivationFunctionType.Exp

    nc = bacc.Bacc("TRN2", target_bir_lowering=False, debug=False)

    xt = nc.dram_tensor("xt", [C, N], bf16, kind="ExternalInput").ap()
    wqkvt = nc.dram_tensor("wqkvt", [C, 3 * C], bf16, kind="ExternalInput").ap()
    wprojt = nc.dram_tensor("wprojt", [C, C], bf16, kind="ExternalInput").ap()
    m1w_in = nc.dram_tensor("m1w", [12 * G, 128], bf16, kind="ExternalInput").ap()
    m2p_in = nc.dram_tensor("m2p", [128, 128], f32, kind="ExternalInput").ap()
    out_d = nc.dram_tensor("out", [C, N], f32, kind="ExternalOutput").ap()

    KC = C // 128  # 6 contraction chunks

    # greedy ACT/DVE balance by estimated occupancy (ns)
    _load = {"act": 0.0, "dve": 0.0}

    def evac(dst, src, n):
        """PSUM->SBUF copy of [rows, n]; pick the less-loaded engine."""
        if _load["act"] + 200 + n / 1.2 <= _load["dve"] + 200 + n / 0.96:
            _load["act"] += 200 + n / 1.2
            nc.scalar.copy(dst, src)
        else:
            _load["dve"] += 200 + n / 0.96
            nc.vector.tensor_copy(dst, src)

    with tile.TileContext(nc) as tc:
        with tc.tile_pool(name="const", bufs=1) as const, \
             tc.tile_pool(name="big", bufs=1) as big:

            m1wsb = const.tile([120, 128], bf16)
            nc.sync.dma_start(m1wsb, m1w_in)
            m2psb = const.tile([128, 128], f32)
            nc.sync.dma_start(m2psb, m2p_in)

            # persistent activations
            qksb = big.tile([128, 2 * KC, N], bf16)   # ct 0..5 = q, 6..11 = k
            vsb = big.tile([128, 8, C], bf16)         # [m%128, m//128, cout]
            attnT = big.tile([128, KC, N], bf16)      # [cout2%128, cout2//128, n]

            # ---------------- QKV ----------------
            with tc.tile_pool(name="xtp", bufs=1) as xtp, \
                 tc.tile_pool(name="qkvps", bufs=3, space="PSUM") as qkvps, \
                 tc.tile_pool(name="vps", bufs=2, space="PSUM") as vps:
                xtsb = xtp.tile([128, KC, N], bf16)
                wqsb = xtp.tile([128, KC, 3 * C], bf16)
                wqr = wqkvt.rearrange("(kc p) c -> p kc c", p=128)
                xtr = xt.rearrange("(kc p) n -> p kc n", p=128)
                # split loads so the first q matmuls start early
                nc.sync.dma_start(wqsb[:, :, 0:C], wqr[:, :, 0:C])
                nc.sync.dma_start(xtsb[:, :, 0:512], xtr[:, :, 0:512])
                nc.sync.dma_start(wqsb[:, :, C:2 * C], wqr[:, :, C:2 * C])
                nc.sync.dma_start(xtsb[:, :, 512:N], xtr[:, :, 512:N])
                nc.sync.dma_start(wqsb[:, :, 2 * C:3 * C], wqr[:, :, 2 * C:3 * C])

                # q, k: out[cout_tile, n]
                for ct in range(12):
                    for nh in range(2):
                        ps = qkvps.tile([128, 512], f32, tag="qkv")
                        for kc in range(KC):
                            nc.tensor.matmul(
                                ps,
                                lhsT=wqsb[:, kc, 128 * ct:128 * ct + 128],
                                rhs=xtsb[:, kc, 512 * nh:512 * nh + 512],
                                start=(kc == 0), stop=(kc == KC - 1),
                            )
                        evac(qksb[:, ct, 512 * nh:512 * nh + 512], ps, 512)

                # v: out[n_tile, cout]
                for nt in range(8):
                    ps = vps.tile([128, 768], f32, tag="vps")
                    for half, (c0, c1) in enumerate([(0, 512), (512, 768)]):
                        for kc in range(KC):
                            nc.tensor.matmul(
                                ps[:, c0:c1],
                                lhsT=xtsb[:, kc, 128 * nt:128 * nt + 128],
                                rhs=wqsb[:, kc, 2 * C + c0:2 * C + c1],
                                start=(kc == 0), stop=(kc == KC - 1),
                            )
                    evac(vsb[:, nt, :], ps, 768)

            # ------- attention: software-pipelined over blocks -------
            # Per iteration (emission order == execution order per engine):
            #   1. dense PE burst: AV chains of block b-2 woven with score
            #      matmul pairs of block b (keeps the PE busy enough for the
            #      HAM clock-gate to hold 2.4 GHz; score evacs drain on
            #      ACT/DVE underneath),
            #   2. interleave DMAs of block b (sync/gpsimd alternating),
            #   3. mix steps of block b-1 (mix2T lags mix1 by SKEW to hide
            #      the ACT exp latency).
            # Mix matmuls carry 128 weight columns so the compiler enables
            # FWL (4x faster LDWEIGHTS): m1w is column-padded with zeros, so
            # a1 rows [rows:128) are written as zeros, exp turns them into
            # finite 1s, and m2p's zero rows [120:128) zero them out of A2.
            blocks = _block_layout()
            nblk = len(blocks)

            with tc.tile_pool(name="sallp", bufs=1) as sallp, \
                 tc.tile_pool(name="sintp", bufs=2) as sintp, \
                 tc.tile_pool(name="pintp", bufs=6) as pintp, \
                 tc.tile_pool(name="a2tp", bufs=2) as a2tp, \
                 tc.tile_pool(name="smp", bufs=8) as smp, \
                 tc.tile_pool(name="m2wp", bufs=6) as m2wp, \
                 tc.tile_pool(name="psmix", bufs=2, space="PSUM") as psmix, \
                 tc.tile_pool(name="psA2", bufs=3, space="PSUM") as psA2, \
                 tc.tile_pool(name="psAV", bufs=1, space="PSUM") as psAV:

                sint_t = {}
                a2t_t = {}
                _dq = [0]

                def s_thunks(bi):
                    """Scores of block bi: 12 per-head thunks (2 MMs into a
                    2-bank psum tile + one evac), then the interleave DMAs."""
                    n0, nb, chunks = blocks[bi]
                    sall = sallp.tile([128, 12, N], bf16, tag="sall",
                                      name="sall")
                    out = []

                    (rs0, g00, gc, gs) = chunks[0]

                    def mk(h):
                        def emit():
                            base = 64 * (h % 2)
                            ps = psmix.tile([128, 1024], f32, tag="ps",
                                            name="ps")
                            # x columns are block-locally permuted on the
                            # host so that psum row rr*gc+g holds query
                            # n0+g*gs+rr: the interleave below then reads a
                            # stride-gc partition slice (spreads across all
                            # SDMA engine ports), and the permutation
                            # cancels at the AV stage.
                            for mh in range(2):
                                nc.tensor.matmul(
                                    ps[0:nb, 512 * mh:512 * mh + 512],
                                    lhsT=qksb[base:base + 64, h // 2,
                                              n0:n0 + nb],
                                    rhs=qksb[base:base + 64, 6 + h // 2,
                                             512 * mh:512 * mh + 512],
                                    start=True, stop=True,
                                )
                            evac(sall[0:nb, h, :], ps[0:nb, :], 1024)
                        return emit

                    for h in range(12):
                        out.append(mk(h))

                    def interleave():
                        # sint[rr*12+h, g, m] = sall[rr*gc+g, h, m]
                        sint = sintp.tile([128, 12, N], bf16, tag="sint",
                                          name="sint")
                        sint_t[bi] = sint
                        for g in range(gc):
                            eng = nc.sync if _dq[0] % 2 == 0 else nc.gpsimd
                            _dq[0] += 1
                            eng.dma_start(
                                out=sint[0:12 * gs, g, :],
                                in_=sall[g:gc * gs:gc, :, :],
                            )
                    out.append(interleave)
                    return out

                def stage_M_gen(bi):
                    """Mixes of block bi at group steps; mix2 (transposed,
                    straight into a2t layout) lags mix1 by SKEW steps."""
                    n0, nb, chunks = blocks[bi]
                    sint = sint_t.pop(bi)
                    a2t = a2tp.tile([128, 12, 8, 128], bf16, tag="a2t",
                                    name="a2t")
                    a2t_t[bi] = a2t
                    steps = []
                    for (rs, g0, gc, gs) in chunks:
                        for g in range(g0, g0 + gc):
                            steps.append((g, gs))
                    st = {}

                    def mix1(g, gs):
                        rows = 12 * gs
                        pg = pintp.tile([128, N], bf16, tag="pint", name="pg")
                        sm = smp.tile([128, 2], f32, tag="sm", name="sm")
                        st[g] = {"pg": pg, "sm": sm, "w2": None, "gs": gs}
                        a1 = psmix.tile([128, 1024], f32, tag="ps", name="a1")
                        for mh in range(2):
                            nc.tensor.matmul(
                                a1[:, 512 * mh:512 * mh + 512],
                                lhsT=m1wsb[0:rows, 0:128],
                                rhs=sint[0:rows, g, 512 * mh:512 * mh + 512],
                                start=True, stop=True,
                            )
                        _load["act"] += 1050
                        nc.scalar.activation(
                            pg, a1, Exp,
                            accum_out=sm[:, 0:1],
                        )
                        _load["dve"] += 400
                        nc.vector.reciprocal(sm[:, 1:2], sm[:, 0:1])
                        w2 = m2wp.tile([128, 128], bf16, tag="m2w", name="w2")
                        st[g]["w2"] = w2
                        kr = 128 if gs == G else 12 * gs
                        # on DVE right behind the reciprocal: gpsimd tensor
                        # ops cost ~2us of Q7 dispatch and sat in the
                        # exp->w2->mix2T critical chain
                        nc.vector.tensor_scalar_mul(
                            w2[0:kr, :], m2psb[0:kr, :], sm[0:kr, 1:2])

                    def mix2T(g, gs):
                        # A2^T chunk: out[m', (rr,o)] = sum_rows
                        #   pg[row, m'] * w2[row, (rr,o)] -- m on partitions,
                        # written directly into the a2t (transposed) layout.
                        # K=128 (padded rows contribute exp(0)*0) -> FWL.
                        s = st.pop(g)
                        kr = 128 if gs == G else 12 * gs
                        for hf in range(2):
                            a2 = psA2.tile([128, 4, 128], f32, tag="psA2",
                                           name="a2")
                            for cc in range(4):
                                c = 4 * hf + cc
                                nc.tensor.matmul(
                                    a2[:, cc, :],
                                    lhsT=s["pg"][0:kr, 128 * c:128 * c + 128],
                                    rhs=s["w2"][0:kr, :],
                                    start=True, stop=True,
                                )
                            evac(a2t[:, g, 4 * hf:4 * hf + 4, :], a2, 512)

                    SKEW = 4
                    for i in range(len(steps) + SKEW):
                        if i < len(steps):
                            mix1(*steps[i])
                        if i >= SKEW:
                            mix2T(*steps[i - SKEW])
                        yield

                def av_thunks(bi):
                    """attention@V of block bi: 6 column-packed head-pair
                    chains accumulating into shared 1-bank psum tiles."""
                    n0, nb, chunks = blocks[bi]
                    a2t = a2t_t.pop(bi)
                    out = []
                    tiles = {}

                    def mk(j):
                        def emit():
                            grp, jj = divmod(j, 4)
                            npair = 4 if grp == 0 else 2
                            if jj == 0:
                                tiles[grp] = psAV.tile(
                                    [128, npair, 128], f32, tag="psAV",
                                    name="av", padded_shape=[128, 4, 128])
                            av = tiles[grp]
                            for half in range(2):
                                o = 2 * j + half
                                for (rs, g0, gc, gs) in chunks:
                                    for c in range(8):
                                        nc.tensor.matmul(
                                            av[64 * half:64 * half + 64, jj,
                                               rs:rs + gc * gs],
                                            lhsT=vsb[:, c, 64 * o:64 * o + 64],
                                            rhs=a2t[:, g0:g0 + gc, c,
                                                    o:o + 12 * (gs - 1) + 1:12],
                                            start=(c == 0), stop=(c == 7),
                                        )
                            if jj == npair - 1:
                                evac(attnT[:, 4 * grp:4 * grp + npair,
                                           n0:n0 + nb],
                                     av[:, :, 0:nb], npair * nb)
                        return emit

                    for j in range(6):
                        out.append(mk(j))
                    return out

                for it in range(nblk + 2):
                    avs = av_thunks(it - 2) if 2 <= it <= nblk + 1 else []
                    ss = s_thunks(it) if it < nblk else []
                    # 1. dense PE burst: AV chains woven with score pairs
                    si = 0
                    for t in avs:
                        t()
                        for _ in range(2):
                            if si < len(ss) - 1:
                                ss[si]()
                                si += 1
                    while si < len(ss) - 1:
                        ss[si]()
                        si += 1
                    # 2. interleave DMAs of block `it`
                    if ss:
                        ss[-1]()
                    # 3. mixes of block it-1
                    if 1 <= it <= nblk:
                        for _ in stage_M_gen(it - 1):
                            pass

            # ---------------- proj ----------------
            with tc.tile_pool(name="projps", bufs=3, space="PSUM") as pjp, \
                 tc.tile_pool(name="outp", bufs=3) as outp:
                wpsb = outp.tile([128, KC, C], bf16)
                nc.sync.dma_start(
                    wpsb, wprojt.rearrange("(kc p) c -> p kc c", p=128))
                od = out_d.rearrange("(ct p) n -> p ct n", p=128)
                for ct in range(KC):
                    for nh in range(2):
                        ps = pjp.tile([128, 512], f32, tag="pj")
                        for kc in range(KC):
                            nc.tensor.matmul(
                                ps,
                                lhsT=wpsb[:, kc, 128 * ct:128 * ct + 128],
                                rhs=attnT[:, kc, 512 * nh:512 * nh + 512],
                                start=(kc == 0), stop=(kc == KC - 1),
                            )
                        ob = outp.tile([128, 512], f32, tag="ob")
                        evac(ob, ps, 512)
                        nc.sync.dma_start(
                            od[:, ct, 512 * nh:512 * nh + 512], ob)

    nc.compile()
    return nc


def _mix_weights(conv_l_w, conv_w_w):
    """Host-built mix lhsT matrices, row map r = rr*12 + h.

    m1w[rr*12+h, rr*12+o] = SCALE * conv_l[o, h]   (lhsT for mix1)
    m2p[rr*12+h, rr*12+o] = conv_w[o, h]           (pattern, f32; scaled
        per-group on device by 1/softmax_sum per row; 128 cols, cols >= 120
        are zero so A2 psum rows [rows:128) are zeros)
    The gs=4 ragged group uses the leading [48, 48] / [48, :] slices.
    """
    m1 = np.zeros((120, 128), np.float32)
    m2 = np.zeros((128, 128), np.float32)
    for rr in range(G):
        for h in range(12):
            for o in range(12):
                m1[rr * 12 + h, rr * 12 + o] = SCALE * conv_l_w[o, h]
                m2[rr * 12 + h, rr * 12 + o] = conv_w_w[o, h]
    return m1.astype(BF16), m2.astype(np.float32)


def _perm():
    """Block-local query permutation: position n0+rr*gc+g holds query
    n0+g*gs+rr, so score-matmul output rows land pre-interleaved."""
    p = np.arange(N)
    for (n0, nb, chunks) in _block_layout():
        (rs, g0, gc, gs) = chunks[0]
        for rr in range(gs):
            for g in range(gc):
                p[n0 + rr * gc + g] = n0 + g * gs + rr
    return p


def _run(x, w_qkv, w_proj, b_proj, conv_l_w, conv_w_w, **spmd_kwargs):
    global _cached
    from concourse import bass_utils

    x = np.asarray(x, np.float32)
    w_qkv = np.asarray(w_qkv, np.float32)
    w_proj = np.asarray(w_proj, np.float32)
    b_proj = np.asarray(b_proj, np.float32)
    conv_l_w = np.asarray(conv_l_w, np.float32)
    conv_w_w = np.asarray(conv_w_w, np.float32)

    if _cached is None:
        _cached = _build_program()
    nc = _cached

    m1w, m2p = _mix_weights(conv_l_w, conv_w_w)
    wqkvt = np.ascontiguousarray(w_qkv.T).astype(BF16)
    wprojt = np.ascontiguousarray(w_proj.T).astype(BF16)

    perm = _perm()
    in_maps = []
    for b in range(B):
        in_maps.append({
            "xt": np.ascontiguousarray(x[b].T[:, perm]).astype(BF16),
            "wqkvt": wqkvt,
            "wprojt": wprojt,
            "m1w": m1w,
            "m2p": m2p,
        })

    res = bass_utils.run_bass_kernel_spmd(
        nc, in_maps, core_ids=list(range(B)), **spmd_kwargs)
    out = np.stack([res.results[b]["out"].T for b in range(B)])  # [B, N, C]
    return (out + b_proj[None, None, :]).astype(np.float32), res


def kernel(x, w_qkv, w_proj, b_proj, conv_l_w, conv_w_w):
    out, _ = _run(x, w_qkv, w_proj, b_proj, conv_l_w, conv_w_w)
    return out
